# revision 34
# baseline (speedup 1.0000x reference)
"""Trainium2 Bass kernel for nn_L2GTraversal (leaf->level1->root point-cloud net).

Strategy (8 NeuronCores, data-parallel over leaves):
  - 64 leaves x 2048 points; core m owns leaves 8m..8m+7 (16384 points).
  - All activations kept TRANSPOSED (channels on partitions, points on the
    free dim) so every layer is lhsT=weight (stationary), rhs=activation^T,
    and the per-leaf max-pool is a free-dim reduce.
  - Algebraic fold: proj@We1[3:] with proj = relu1@Wp2 + bp2 is folded to
    relu1@(Wp2@We1[3:]) + const-bias, removing one 128x128 GEMM per point.
  - relu/max/bias commute: the last-layer relu+bias is applied after the
    per-leaf max-pool (on 512 values/leaf instead of 2048x512).
  - Encoder GEMMs run in bf16 (full-rate PE at any tile size, half the DMA
    bytes); PSUM accumulation stays f32.  Verified fro rel err ~2e-3 vs the
    f32 reference (threshold 2e-2).
  - Software pipeline (chunk = 256 points): PE does [psE1(q), mm1(q+1),
    psE2(q-1)] per iteration so every matmul's operands were produced at
    least one iteration earlier -> no PE stalls.  Act runs [relu1(q+1),
    h0(q), h1(q)].  The four psE2 accumulators of one chunk live in a single
    (128,4,256) PSUM tile so the max-pool drain is ONE vector reduce per
    chunk, double-buffered across iterations.
  - DMA count is minimized (the HWDGE has a ~625ns fixed cost per transfer):
    all encoder weights ride in one packed tensor, all biases in another,
    per-leaf point features and relative coords ride together in one 35-row
    tensor (rel rows at partitions 32:35, matched by we1a packed at the same
    base partition), and the whole output leaves in ONE DMA from a staging
    tile.
  - Level-1 aggregation is core-local (leaves 8m..8m+7 are exactly parent
    m's children).  The cross-core max for the root row is done on the HOST:
    each core outputs g2_m = relu(Wa1^T [level1_m; relpos]); the host takes
    the elementwise max over the 8 cores and applies the final 512x512 GEMV
    (0.5 MFLOP).  This removes the device AllReduce from the critical path.

Host side does only: index gathers, transposes/packing for the chosen
sharding layout, the one-time weight fold, dtype conversion, the tiny root
epilogue, and output reassembly.
"""

import numpy as np

import ml_dtypes

import concourse.bass as bass  # noqa: F401
import concourse.mybir as mybir
import concourse.tile as tile
from concourse import bacc
from concourse.bass_utils import run_bass_kernel_spmd

NCORES = 8
L, K, C = 64, 2048, 32
LPC = L // NCORES            # leaves per core
PTS = LPC * K                # points per core
D_PROJ, D_HID, D = 128, 256, 512
CH = 256                     # point-chunk (matmul free dim)
CPL = K // CH                # chunks per leaf
NQ = LPC * CPL               # chunks per core
LAG = 1                      # psE2 trails psE1 by this many chunks
OC = LPC + 2                 # output columns per o2-block
F32 = mybir.dt.float32
BF16 = mybir.dt.bfloat16
RELU = mybir.ActivationFunctionType.Relu


def _bf16(a):
    return np.ascontiguousarray(
        np.ascontiguousarray(a, np.float32).astype(ml_dtypes.bfloat16))


def _emit(tc, tin, tout, reps=1):
    nc = tc.nc
    import contextlib

    ctx = contextlib.ExitStack()
    with ctx:
        const = ctx.enter_context(tc.tile_pool(name="const", bufs=1))
        io = ctx.enter_context(tc.tile_pool(name="io", bufs=1))
        act = ctx.enter_context(tc.tile_pool(name="act", bufs=1))
        red = ctx.enter_context(tc.tile_pool(name="red", bufs=1))
        agg = ctx.enter_context(tc.tile_pool(name="agg", bufs=1))
        psp = ctx.enter_context(tc.tile_pool(name="psum", bufs=1, space="PSUM"))

        cw = {}
        for rep in range(reps):
            _emit_body(nc, tc, tin, tout, const, io, act, red, agg, psp, cw,
                       first=(rep == 0))


def _emit_body(nc, tc, tin, tout, const, io, act, red, agg, psp, cw, first):
    def cdma(name, shape, dt=F32):
        t = const.tile(list(shape), dt, name=name, tag=name)
        nc.sync.dma_start(out=t, in_=tin[name][:, :])
        cw[name] = t
        return t
    xTd = tin["xT"]

    # leaf-feature max accumulators: lfm[ch, o2, leaf] (pre-bias, pre-relu)
    lfm = red.tile([128, 4, LPC], F32, name="lfm", tag="lfm")

    xT = {}      # leaf -> (35, K) sbuf tile (bf16): rows 0:32 pf, 32:35 rel
    mxp = {}     # leaf -> (128, 4, CPL) chunk-max tile
    ps1s = {}    # chunk -> psum tile
    relu1s = {}  # chunk -> sbuf bf16 tile
    psE1s = {}   # chunk -> psum tile (128, 2, CH)
    hs = {}      # (chunk, ot) -> sbuf bf16 tile
    psE2s = {}   # chunk -> psum tile (128, 4, CH)

    def load_leaf(l):
        t = io.tile([35, K], BF16, name=f"xT_l{l}", tag="xT", bufs=3)
        nc.sync.dma_start(out=t, in_=xTd[:, l * K:(l + 1) * K])
        xT[l] = t
        mxp[l] = red.tile([128, 4, CPL], F32, name=f"mx_l{l}", tag="mx",
                          bufs=2)

    def emit_mm1(q):
        l, c = q // CPL, q % CPL
        ps1 = psp.tile([128, CH], F32, name=f"ps1_q{q}", tag="ps1", bufs=2)
        nc.tensor.matmul(ps1, wp1t, xT[l][0:32, c * CH:(c + 1) * CH],
                         start=True, stop=True)
        ps1s[q] = ps1

    def emit_relu1(q):
        r = act.tile([128, CH], BF16, name=f"relu1_q{q}", tag="relu1", bufs=3)
        nc.scalar.activation(r, ps1s[q], RELU, bias=bp1)
        del ps1s[q]
        relu1s[q] = r

    def emit_psE1(q):
        l, c = q // CPL, q % CPL
        p = psp.tile([128, 2, CH], F32, name=f"psE1_q{q}", tag="psE1", bufs=2)
        rT = xT[l][32:35, c * CH:(c + 1) * CH]
        for ot in range(2):
            sl = slice(ot * 128, (ot + 1) * 128)
            nc.tensor.matmul(p[:, ot, :], w2e[:, sl], relu1s[q],
                             start=True, stop=False)
            nc.tensor.matmul(p[:, ot, :], we1a[:, sl], rT,
                             start=False, stop=True)
        del relu1s[q]
        psE1s[q] = p

    def emit_h(q):
        p = psE1s[q]
        for ot in range(2):
            h = act.tile([128, CH], BF16, name=f"hT_q{q}_{ot}",
                         tag=f"hT{ot}", bufs=LAG + 1)
            nc.scalar.activation(h, p[:, ot, :], RELU, bias=be1f[:, ot:ot + 1])
            hs[(q, ot)] = h
        del psE1s[q]

    def emit_psE2(q):
        p2 = psp.tile([128, 4, CH], F32, name=f"psE2_q{q}", tag="psE2",
                      bufs=2)
        for o2 in range(4):
            sl = slice(o2 * 128, (o2 + 1) * 128)
            nc.tensor.matmul(p2[:, o2, :], we2[0][:, sl], hs[(q, 0)],
                             start=True, stop=False)
            nc.tensor.matmul(p2[:, o2, :], we2[1][:, sl], hs[(q, 1)],
                             start=False, stop=True)
        del hs[(q, 0)], hs[(q, 1)]
        psE2s[q] = p2

    def emit_drain(q):
        l, c = q // CPL, q % CPL
        nc.vector.reduce_max(out=mxp[l][:, :, c], in_=psE2s[q],
                             axis=mybir.AxisListType.X)
        del psE2s[q]

    def leaf_final(l):
        nc.vector.reduce_max(out=lfm[:, :, l], in_=mxp[l],
                             axis=mybir.AxisListType.X)
        del mxp[l], xT[l]

    # ---- load order: a small tensor with wp1 + all biases first (mm1 and
    # relu1 can start ~2us in), then leaf 0, then the big encoder weights,
    # then the aggregation weights, then the leaf prefetches from the loop.
    if first:
        cdma("wsmall", (128, 158), BF16)
    load_leaf(0)
    if first:
        cdma("wall", (128, 1536), BF16)
    load_leaf(1)
    if first:
        cdma("wagg", (128, 4096))
        cdma("waggr", (3, 512 + LPC + 1))
    wall = cw["wall"]
    wsmall = cw["wsmall"]
    wp1t = wsmall[0:32, 30:158]
    biases = wsmall[:, 0:30].bitcast(F32)
    bp1 = biases[:, 0:1]
    be1f = biases[:, 1:3]
    be2c = biases[:, 3:7]
    ba1c = biases[:, 7:11]
    ba2c = biases[:, 11:15]
    w2e = wall[:, 0:256]
    we1a = wall[32:35, 256:512]
    we2 = [wall[:, 512:1024], wall[:, 1024:1536]]
    wagg, waggr = cw["wagg"], cw["waggr"]  # noqa: F841
    wa1 = [wagg[:, kt * 512:(kt + 1) * 512] for kt in range(4)]
    wa2 = [wagg[:, 2048 + kt * 512:2048 + (kt + 1) * 512] for kt in range(4)]
    wa1r = waggr[:, 0:512]
    relcT = waggr[:, 512:512 + LPC]
    r2T = waggr[:, 512 + LPC:512 + LPC + 1]

    def post_chunk(qq):
        emit_psE2(qq)
        emit_drain(qq)
        if qq % CPL == CPL - 1:
            leaf_final(qq // CPL)

    emit_mm1(0)
    emit_relu1(0)
    for q in range(NQ):
        l, c = q // CPL, q % CPL
        if c == 0 and l + 2 < LPC:
            load_leaf(l + 2)
        if q + 1 < NQ:
            emit_mm1(q + 1)
            emit_relu1(q + 1)
        emit_psE1(q)
        emit_h(q)
        if q >= LAG:
            post_chunk(q - LAG)
    for qq in range(NQ - LAG, NQ):
        post_chunk(qq)

    # ---- output staging: leaf block leaves early, lvl1+g2 in a second DMA
    ostl = agg.tile([128, 4, LPC], F32, name="ostl", tag="ostl")
    osts = agg.tile([128, 4, 2], F32, name="osts", tag="osts")

    # leaf features: bias + relu into staging cols 0..LPC
    for o2 in range(4):
        nc.scalar.activation(ostl[:, o2, :], lfm[:, o2, :], RELU,
                             bias=be2c[:, o2:o2 + 1])
    nc.sync.dma_start(out=tout[:, 0:4 * LPC], in_=ostl)

    # level 1: g1 = relu(Wa1^T [leaf_feat; relc] + ba1); max; @Wa2 + ba2
    m1 = []
    for o2 in range(4):
        sl = slice(o2 * 128, (o2 + 1) * 128)
        pt = psp.tile([128, CH], F32, name=f"psA{o2}", tag="ps1", bufs=2)
        psA = pt[:, 0:LPC]
        for kt in range(4):
            nc.tensor.matmul(psA, wa1[kt][:, sl], ostl[:, kt, :],
                             start=(kt == 0), stop=False)
        nc.tensor.matmul(psA, wa1r[:, sl], relcT, start=False, stop=True)
        g1 = agg.tile([128, LPC], F32, name=f"g1_{o2}", tag=f"g1_{o2}")
        nc.scalar.activation(g1, psA, RELU, bias=ba1c[:, o2:o2 + 1])
        m = agg.tile([128, 1], F32, name=f"m1_{o2}", tag=f"m1_{o2}")
        nc.vector.reduce_max(out=m, in_=g1, axis=mybir.AxisListType.X)
        m1.append(m)

    for o2 in range(4):
        sl = slice(o2 * 128, (o2 + 1) * 128)
        pt = psp.tile([128, CH], F32, name=f"psL{o2}", tag="ps1", bufs=2)
        psL = pt[:, 0:1]
        for kt in range(4):
            nc.tensor.matmul(psL, wa2[kt][:, sl], m1[kt],
                             start=(kt == 0), stop=(kt == 3))
        nc.scalar.add(osts[:, o2, 0:1], psL, ba2c[:, o2:o2 + 1])

    # root partial: g2 = relu(Wa1^T [lvl1; r2] + ba1) -> host epilogue
    for o2 in range(4):
        sl = slice(o2 * 128, (o2 + 1) * 128)
        pt = psp.tile([128, CH], F32, name=f"psR{o2}", tag="ps1", bufs=2)
        psR = pt[:, 0:1]
        for kt in range(4):
            nc.tensor.matmul(psR, wa1[kt][:, sl], osts[:, kt, 0:1],
                             start=(kt == 0), stop=False)
        nc.tensor.matmul(psR, wa1r[:, sl], r2T, start=False, stop=True)
        nc.scalar.activation(osts[:, o2, 1:2], psR, RELU,
                             bias=ba1c[:, o2:o2 + 1])
    nc.sync.dma_start(out=tout[:, 4 * LPC:4 * LPC + 8], in_=osts)


_CACHE = {}


def _build(reps=1):
    key = ("nc", reps)
    if key in _CACHE:
        return _CACHE[key]
    nc = bacc.Bacc("TRN2", target_bir_lowering=False, debug=False,
                   num_devices=NCORES)
    shapes = {
        "xT": ((35, PTS), BF16),
        "wsmall": ((128, 158), BF16),
        "wall": ((128, 1536), BF16),
        "wagg": ((128, 4096), F32),
        "waggr": ((3, 512 + LPC + 1), F32),
    }
    tin = {name: nc.dram_tensor(name, list(shape), dt,
                                kind="ExternalInput").ap()
           for name, (shape, dt) in shapes.items()}
    tout = nc.dram_tensor("out", [128, 4 * OC], F32, kind="ExternalOutput").ap()
    with tile.TileContext(nc) as tc:
        _emit(tc, tin, tout, reps=reps)
    nc.compile()
    _CACHE[key] = nc
    return nc


def _prep_in_maps(inputs):
    f32 = np.float32
    coords = np.asarray(inputs["coords"], f32)
    feats = np.asarray(inputs["feats"], f32)
    leaf_indices = np.asarray(inputs["leaf_indices"])
    leaf_center_idx = np.asarray(inputs["leaf_center_idx"])
    l1_center_idx = np.asarray(inputs["l1_center_idx"])
    root_center_idx = int(np.asarray(inputs["root_center_idx"]))

    pts = coords[leaf_indices]            # (L, K, 3)
    pf = feats[leaf_indices]              # (L, K, C)
    centers = coords[leaf_center_idx]     # (L, 3)
    pp = coords[l1_center_idx]            # (B1, 3)
    rootc = coords[root_center_idx]       # (3,)

    Wp1 = np.asarray(inputs["Wp1"], f32)
    bp1 = np.asarray(inputs["bp1"], f32)
    Wp2 = np.asarray(inputs["Wp2"], f32)
    bp2 = np.asarray(inputs["bp2"], f32)
    We1 = np.asarray(inputs["We1"], f32)
    be1 = np.asarray(inputs["be1"], f32)
    We2 = np.asarray(inputs["We2"], f32)
    be2 = np.asarray(inputs["be2"], f32)
    Wa1 = np.asarray(inputs["Wa1"], f32)
    ba1 = np.asarray(inputs["ba1"], f32)
    Wa2 = np.asarray(inputs["Wa2"], f32)
    ba2 = np.asarray(inputs["ba2"], f32)

    # fold proj's second linear into the encoder first layer (fp64 for safety)
    We1a = np.ascontiguousarray(We1[0:3])                       # (3, 256)
    We1b = We1[3:131]                                           # (128, 256)
    W2e = (Wp2.astype(np.float64) @ We1b.astype(np.float64)).astype(f32)
    be1f = (be1.astype(np.float64)
            + bp2.astype(np.float64) @ We1b.astype(np.float64)).astype(f32)

    biases = np.zeros((128, 15), f32)
    biases[:, 0] = bp1
    biases[:, 1:3] = be1f.reshape(2, 128).T
    biases[:, 3:7] = be2.reshape(4, 128).T
    biases[:, 7:11] = ba1.reshape(4, 128).T
    biases[:, 11:15] = ba2.reshape(4, 128).T

    # small fast-start tensor: f32 biases bitcast into 30 bf16 cols + wp1
    wsmall = np.zeros((128, 158), ml_dtypes.bfloat16)
    wsmall[:, 0:30] = np.ascontiguousarray(biases).view(
        np.uint16).view(ml_dtypes.bfloat16)
    wsmall[0:32, 30:158] = _bf16(Wp1)
    # packed encoder weights: bf16 cols [w2e | we1a | we2_0 | we2_1]
    wall = np.zeros((128, 1536), ml_dtypes.bfloat16)
    wall[:, 0:256] = _bf16(W2e)
    wall[32:35, 256:512] = _bf16(We1a)
    wall[:, 512:1024] = _bf16(We2[0:128])
    wall[:, 1024:1536] = _bf16(We2[128:256])

    wagg = np.zeros((128, 4096), f32)
    for kt in range(4):
        wagg[:, kt * 512:(kt + 1) * 512] = Wa1[kt * 128:(kt + 1) * 128]
        wagg[:, 2048 + kt * 512:2048 + (kt + 1) * 512] = \
            Wa2[kt * 128:(kt + 1) * 128]

    common = {
        "wsmall": np.ascontiguousarray(wsmall),
        "wall": np.ascontiguousarray(wall),
        "wagg": np.ascontiguousarray(wagg),
    }

    in_maps = []
    for m in range(NCORES):
        sl = slice(m * LPC, (m + 1) * LPC)
        im = dict(common)
        xT = np.empty((35, PTS), f32)
        xT[0:32] = pf[sl].reshape(PTS, C).T
        rel = pts[sl] - centers[sl][:, None, :]                 # (LPC, K, 3)
        xT[32:35] = rel.reshape(PTS, 3).T
        im["xT"] = _bf16(xT)
        waggr = np.zeros((3, 512 + LPC + 1), f32)
        waggr[:, 0:512] = Wa1[512:515]
        waggr[:, 512:512 + LPC] = (centers[sl] - pp[m]).T
        waggr[:, 512 + LPC] = pp[m] - rootc
        im["waggr"] = waggr
        in_maps.append(im)
    return in_maps


def _run(inputs, **kwargs):
    nc = _build()
    in_maps = _prep_in_maps(inputs)
    res = run_bass_kernel_spmd(nc, in_maps, core_ids=list(range(NCORES)),
                               **kwargs)
    out = np.empty((1 + NCORES + L, D), np.float32)
    g2 = np.empty((NCORES, D), np.float32)
    for m in range(NCORES):
        o = res.results[m]["out"]                     # (128, 4*LPC + 8)
        leaf = o[:, 0:4 * LPC].reshape(128, 4, LPC)
        out[1 + NCORES + m * LPC:1 + NCORES + (m + 1) * LPC] = \
            leaf.transpose(1, 0, 2).reshape(D, LPC).T
        small = o[:, 4 * LPC:].reshape(128, 4, 2)
        out[1 + m] = small[:, :, 0].T.reshape(D)
        g2[m] = small[:, :, 1].T.reshape(D)
    # root epilogue on host: elementwise max of the per-core g2 partials,
    # then the final 512x512 GEMV.
    Wa2 = np.asarray(inputs["Wa2"], np.float32)
    ba2 = np.asarray(inputs["ba2"], np.float32)
    out[0] = g2.max(axis=0) @ Wa2 + ba2
    return out, res


def kernel(**inputs) -> np.ndarray:
    out, _ = _run(inputs)
    return out


# ---------------------------------------------------------------------------
# dev-only timing helpers (not used by kernel()); safe to keep — they only
# run when called explicitly from test.py.
# ---------------------------------------------------------------------------

def _make_call(nc, in_maps):
    """Build a one-shot timed executor for a compiled bass program."""
    import time

    import jax
    from jax.experimental.shard_map import shard_map
    from jax.sharding import Mesh, NamedSharding, PartitionSpec

    from concourse.bass2jax import (_bass_exec_p, install_neuronx_cc_hook,
                                    partition_id_tensor)

    install_neuronx_cc_hook()
    pname = nc.partition_id_tensor.name if nc.partition_id_tensor else None
    in_names, out_names, out_avals, zero_outs = [], [], [], []
    for alloc in nc.m.functions[0].allocations:
        if not isinstance(alloc, mybir.MemoryLocationSet):
            continue
        name = alloc.memorylocations[0].name
        if alloc.kind == "ExternalInput":
            if name != pname:
                in_names.append(name)
        elif alloc.kind == "ExternalOutput":
            out_names.append(name)
            shape = tuple(alloc.tensor_shape)
            dtype = mybir.dt.np(alloc.dtype)
            out_avals.append(jax.core.ShapedArray(shape, dtype))
            zero_outs.append(np.zeros(shape, dtype))
    n_params = len(in_names)
    all_names = in_names + out_names
    if pname is not None:
        all_names = all_names + [pname]

    def _body(*args):
        operands = list(args)
        if pname is not None:
            operands.append(partition_id_tensor())
        outs = _bass_exec_p.bind(
            *operands, out_avals=tuple(out_avals), in_names=tuple(all_names),
            out_names=tuple(out_names), lowering_input_output_aliases=(),
            sim_require_finite=True, sim_require_nnan=True, nc=nc)
        return tuple(outs)

    ncores = len(in_maps)
    devices = jax.devices()[:ncores]
    mesh = Mesh(np.asarray(devices), ("core",))
    spec = PartitionSpec("core")
    donate = tuple(range(n_params, n_params + len(out_names)))
    fn = jax.jit(
        shard_map(_body, mesh=mesh,
                  in_specs=(spec,) * (n_params + len(out_names)),
                  out_specs=(spec,) * len(out_names), check_rep=False),
        donate_argnums=donate, keep_unused=True)
    sh = NamedSharding(mesh, spec)
    ins = [jax.device_put(
        np.concatenate([np.asarray(m[n]) for m in in_maps], axis=0), sh)
        for n in in_names]
    zs_proto = [np.zeros((ncores * z.shape[0], *z.shape[1:]), z.dtype)
                for z in zero_outs]

    def call():
        zs = [jax.device_put(z, sh) for z in zs_proto]
        jax.block_until_ready(zs)
        t0 = time.perf_counter()
        outs = fn(*ins, *zs)
        # block_until_ready alone does NOT wait for device execution on the
        # axon PJRT stack — a real output fetch does.
        np.asarray(outs[0])
        return time.perf_counter() - t0

    return call


def _pjrt_loop(nc, in_maps, iters):
    call = _make_call(nc, in_maps)
    call()
    return [call() for _ in range(iters)]


def _time_hw(inputs, iters=20, reps=1):
    nc = _build(reps=reps)
    in_maps = _prep_in_maps(inputs)
    return _pjrt_loop(nc, in_maps, iters)


def _time_paired(inputs, reps=10, pairs=100):
    """Per-kernel device time via in-program repetition.

    The e2e call latency here is ~80ms of axon-tunnel overhead with
    multi-ms drift, so a single differential is meaningless.  Instead run
    the 1-rep and `reps`-rep programs interleaved and take the median of
    the per-pair time differences; drift cancels pairwise and the median
    rejects outliers.  Returns (per_rep_ns, pair_diffs_ns).
    """
    import numpy as _np

    in_maps = _prep_in_maps(inputs)
    c1 = _make_call(_build(reps=1), in_maps)
    cR = _make_call(_build(reps=reps), in_maps)
    c1(); cR(); c1(); cR()
    diffs = []
    for i in range(pairs):
        if i % 2 == 0:
            a = c1(); b = cR()
        else:
            b = cR(); a = c1()
        diffs.append((b - a) * 1e9)
    diffs = _np.array(diffs)
    per_rep = float(_np.median(diffs)) / (reps - 1)
    return per_rep, diffs


def _build_baseline():
    if "base" in _CACHE:
        return _CACHE["base"]
    nc = bacc.Bacc("TRN2", target_bir_lowering=False, debug=False,
                   num_devices=NCORES)
    tin = nc.dram_tensor("bx", [128, 4], F32, kind="ExternalInput").ap()
    tout = nc.dram_tensor("bout", [128, 4], F32, kind="ExternalOutput").ap()
    with tile.TileContext(nc) as tc:
        with tc.tile_pool(name="p", bufs=1) as p:
            t = p.tile([128, 4], F32, name="bt", tag="bt")
            nc.sync.dma_start(out=t, in_=tin)
            nc.sync.dma_start(out=tout, in_=t)
    nc.compile()
    _CACHE["base"] = nc
    return nc


def _time_baseline(iters=20):
    nc = _build_baseline()
    in_maps = [{"bx": np.ones((128, 4), np.float32)} for _ in range(NCORES)]
    return _pjrt_loop(nc, in_maps, iters)


# revision 48
# speedup vs baseline: 1.2527x; 1.2527x over previous
"""Trainium2 Bass kernel for nn_L2GTraversal (leaf->level1->root point-cloud net).

Strategy (8 NeuronCores, data-parallel over leaves):
  - 64 leaves x 2048 points; core m owns leaves 8m..8m+7 (16384 points).
  - All activations kept TRANSPOSED (channels on partitions, points on the
    free dim) so every layer is lhsT=weight (stationary), rhs=activation^T,
    and the per-leaf max-pool is a free-dim reduce.
  - Algebraic fold: proj@We1[3:] with proj = relu1@Wp2 + bp2 is folded to
    relu1@(Wp2@We1[3:]) + const-bias, removing one 128x128 GEMM per point.
  - relu/max/bias commute: the last-layer relu+bias is applied after the
    per-leaf max-pool (on 512 values/leaf instead of 2048x512).
  - Encoder GEMMs run in bf16 (full-rate PE at any tile size, half the DMA
    bytes); PSUM accumulation stays f32.  Verified fro rel err ~2e-3 vs the
    f32 reference (threshold 2e-2).
  - Software pipeline (chunk = 256 points): PE does [mm1(q+1), psE1(q),
    psE2(q-1)] per iteration so every matmul's operands were produced at
    least one iteration earlier -> no PE stalls.  Act runs [relu1(q+1),
    h0(q), h1(q)].  The four psE2 accumulators of one chunk live in a single
    (128,4,256) PSUM tile so the max-pool drain is ONE vector reduce per
    chunk, double-buffered across iterations.  Dummy matmuls on a memset
    tile warm the PE p-state while the first DMAs are in flight.
  - DMA count is minimized (the HWDGE has a ~625ns fixed cost per transfer):
    wp1 + all biases ride in a small fast-start tensor (f32 biases bitcast
    into bf16 columns), the remaining encoder weights in two tensors split
    by first use, per-leaf point features and relative coords together in
    one 35-row tensor (rel rows at partitions 32:35, matched by we1a packed
    at the same base partition), and the output leaves in two staged DMAs
    (the big leaf block departs before the aggregation phase runs).
  - The aggregation stages spread their four PSUM groups across the ps1 and
    psE1 bank sets (idle by then) so no group waits on a bank still being
    read by the previous group's activation.
  - Level-1 aggregation is core-local (leaves 8m..8m+7 are exactly parent
    m's children).  The cross-core max for the root row is done on the HOST:
    each core outputs g2_m = relu(Wa1^T [level1_m; relpos]); the host takes
    the elementwise max over the 8 cores and applies the final 512x512 GEMV
    (0.5 MFLOP).  This removes the device AllReduce from the critical path.

Host side does only: index gathers, transposes/packing for the chosen
sharding layout, the one-time weight fold, dtype conversion, the tiny root
epilogue, and output reassembly.
"""

import numpy as np

import ml_dtypes

import concourse.bass as bass  # noqa: F401
import concourse.mybir as mybir
import concourse.tile as tile
from concourse import bacc
from concourse.bass_utils import run_bass_kernel_spmd

NCORES = 8
L, K, C = 64, 2048, 32
LPC = L // NCORES            # leaves per core
PTS = LPC * K                # points per core
D_PROJ, D_HID, D = 128, 256, 512
CH = 256                     # point-chunk (matmul free dim)
CPL = K // CH                # chunks per leaf
NQ = LPC * CPL               # chunks per core
LAG = 1                      # psE2 trails psE1 by this many chunks
OC = LPC + 2                 # output columns per o2-block
F32 = mybir.dt.float32
BF16 = mybir.dt.bfloat16
RELU = mybir.ActivationFunctionType.Relu


def _bf16(a):
    return np.ascontiguousarray(
        np.ascontiguousarray(a, np.float32).astype(ml_dtypes.bfloat16))


def _emit(tc, tin, tout, reps=1):
    nc = tc.nc
    import contextlib

    ctx = contextlib.ExitStack()
    with ctx:
        const = ctx.enter_context(tc.tile_pool(name="const", bufs=1))
        io = ctx.enter_context(tc.tile_pool(name="io", bufs=1))
        act = ctx.enter_context(tc.tile_pool(name="act", bufs=1))
        red = ctx.enter_context(tc.tile_pool(name="red", bufs=1))
        agg = ctx.enter_context(tc.tile_pool(name="agg", bufs=1))
        psp = ctx.enter_context(tc.tile_pool(name="psum", bufs=1, space="PSUM"))

        cw = {}
        for rep in range(reps):
            _emit_body(nc, tc, tin, tout, const, io, act, red, agg, psp, cw,
                       first=(rep == 0))


def _emit_body(nc, tc, tin, tout, const, io, act, red, agg, psp, cw, first):
    def cdma(name, shape, dt=F32):
        t = const.tile(list(shape), dt, name=name, tag=name)
        nc.sync.dma_start(out=t, in_=tin[name][:, :])
        cw[name] = t
        return t
    xTd = tin["xT"]

    # leaf-feature max accumulators: lfm[ch, o2, leaf] (pre-bias, pre-relu)
    lfm = red.tile([128, 4, LPC], F32, name="lfm", tag="lfm")

    xT = {}      # leaf -> (35, K) sbuf tile (bf16): rows 0:32 pf, 32:35 rel
    mxp = {}     # leaf -> (128, 4, CPL) chunk-max tile
    ps1s = {}    # chunk -> psum tile
    relu1s = {}  # chunk -> sbuf bf16 tile
    psE1s = {}   # chunk -> psum tile (128, 2, CH)
    hs = {}      # (chunk, ot) -> sbuf bf16 tile
    psE2s = {}   # chunk -> psum tile (128, 4, CH)

    def load_leaf(l):
        t = io.tile([35, K], BF16, name=f"xT_l{l}", tag="xT", bufs=3)
        nc.sync.dma_start(out=t, in_=xTd[:, l * K:(l + 1) * K])
        xT[l] = t
        mxp[l] = red.tile([128, 4, CPL], F32, name=f"mx_l{l}", tag="mx",
                          bufs=2)

    def emit_mm1(q):
        l, c = q // CPL, q % CPL
        ps1 = psp.tile([128, CH], F32, name=f"ps1_q{q}", tag="ps1", bufs=2)
        nc.tensor.matmul(ps1, wp1t, xT[l][0:32, c * CH:(c + 1) * CH],
                         start=True, stop=True)
        ps1s[q] = ps1

    def emit_relu1(q):
        r = act.tile([128, CH], BF16, name=f"relu1_q{q}", tag="relu1", bufs=3)
        nc.scalar.activation(r, ps1s[q], RELU, bias=bp1)
        del ps1s[q]
        relu1s[q] = r

    def emit_psE1(q):
        l, c = q // CPL, q % CPL
        p = psp.tile([128, 2, CH], F32, name=f"psE1_q{q}", tag="psE1", bufs=2)
        rT = xT[l][32:35, c * CH:(c + 1) * CH]
        for ot in range(2):
            sl = slice(ot * 128, (ot + 1) * 128)
            nc.tensor.matmul(p[:, ot, :], w2e[:, sl], relu1s[q],
                             start=True, stop=False)
            nc.tensor.matmul(p[:, ot, :], we1a[:, sl], rT,
                             start=False, stop=True)
        del relu1s[q]
        psE1s[q] = p

    def emit_h(q):
        p = psE1s[q]
        for ot in range(2):
            h = act.tile([128, CH], BF16, name=f"hT_q{q}_{ot}",
                         tag=f"hT{ot}", bufs=LAG + 1)
            nc.scalar.activation(h, p[:, ot, :], RELU, bias=be1f[:, ot:ot + 1])
            hs[(q, ot)] = h
        del psE1s[q]

    def emit_psE2(q):
        p2 = psp.tile([128, 4, CH], F32, name=f"psE2_q{q}", tag="psE2",
                      bufs=2)
        for o2 in range(4):
            sl = slice(o2 * 128, (o2 + 1) * 128)
            nc.tensor.matmul(p2[:, o2, :], we2[0][:, sl], hs[(q, 0)],
                             start=True, stop=False)
            nc.tensor.matmul(p2[:, o2, :], we2[1][:, sl], hs[(q, 1)],
                             start=False, stop=True)
        del hs[(q, 0)], hs[(q, 1)]
        psE2s[q] = p2

    def emit_drain(q):
        l, c = q // CPL, q % CPL
        nc.vector.reduce_max(out=mxp[l][:, :, c], in_=psE2s[q],
                             axis=mybir.AxisListType.X)
        del psE2s[q]

    def leaf_final(l):
        nc.vector.reduce_max(out=lfm[:, :, l], in_=mxp[l],
                             axis=mybir.AxisListType.X)
        del mxp[l], xT[l]

    # ---- load order: a small tensor with wp1 + all biases first (mm1 and
    # relu1 can start ~2us in), then leaf 0, then the big encoder weights,
    # then the aggregation weights, then the leaf prefetches from the loop.
    if first:
        cdma("wsmall", (128, 158), BF16)
    load_leaf(0)
    if first:
        cdma("wallA", (128, 512), BF16)   # w2e | we1a — needed by psE1(0)
        cdma("wallB", (128, 1024), BF16)  # we2 — needed a chunk later
    load_leaf(1)
    if first:
        cdma("wagg", (128, 4096))
        cdma("waggr", (3, 512 + LPC + 1))
    wsmall = cw["wsmall"]
    wp1t = wsmall[0:32, 30:158]
    biases = wsmall[:, 0:30].bitcast(F32)
    bp1 = biases[:, 0:1]
    be1f = biases[:, 1:3]
    be2c = biases[:, 3:7]
    ba1c = biases[:, 7:11]
    ba2c = biases[:, 11:15]
    w2e = cw["wallA"][:, 0:256]
    we1a = cw["wallA"][32:35, 256:512]
    we2 = [cw["wallB"][:, 0:512], cw["wallB"][:, 512:1024]]
    wagg, waggr = cw["wagg"], cw["waggr"]  # noqa: F841
    wa1 = [wagg[:, kt * 512:(kt + 1) * 512] for kt in range(4)]
    wa2 = [wagg[:, 2048 + kt * 512:2048 + (kt + 1) * 512] for kt in range(4)]
    wa1r = waggr[:, 0:512]
    relcT = waggr[:, 512:512 + LPC]
    r2T = waggr[:, 512 + LPC:512 + LPC + 1]

    def post_chunk(qq):
        emit_psE2(qq)
        emit_drain(qq)
        if qq % CPL == CPL - 1:
            leaf_final(qq // CPL)

    if first:
        # PE p-state warmup: the tensor engine reaches full clock only after
        # ~3us of continuous execution.  While the first weight/leaf DMAs are
        # still in flight, chew through dummy matmuls on a memset tile so the
        # ramp happens before real work arrives.
        warm = io.tile([128, CH], BF16, name="warm", tag="warm")
        nc.gpsimd.memset(warm, 0.0)
        for i in range(14):
            pw = psp.tile([128, CH], F32, name=f"pswarm{i}", tag="ps1",
                          bufs=2)
            nc.tensor.matmul(pw, warm[:, 0:128], warm, start=True, stop=True)

    emit_mm1(0)
    emit_relu1(0)
    for q in range(NQ):
        l, c = q // CPL, q % CPL
        if c == 0 and l + 2 < LPC:
            load_leaf(l + 2)
        if q + 1 < NQ:
            emit_mm1(q + 1)
            emit_relu1(q + 1)
        emit_psE1(q)
        emit_h(q)
        if q >= LAG:
            post_chunk(q - LAG)
    for qq in range(NQ - LAG, NQ):
        post_chunk(qq)

    def _agg_psum(name, o2):
        # spread the four o2 groups of each aggregation stage across the
        # (now idle) ps1 AND psE1 bank sets so no group waits on a bank
        # still being read by the previous group's activation.
        if o2 % 2 == 0:
            return psp.tile([128, CH], F32, name=name, tag="ps1", bufs=2)
        t = psp.tile([128, 2, CH], F32, name=name, tag="psE1", bufs=2)
        return t[:, 0, :]

    # ---- output staging: leaf block leaves early, lvl1+g2 in a second DMA
    ostl = agg.tile([128, 4, LPC], F32, name="ostl", tag="ostl")
    osts = agg.tile([128, 4, 2], F32, name="osts", tag="osts")

    # leaf features: bias + relu into staging cols 0..LPC
    for o2 in range(4):
        nc.scalar.activation(ostl[:, o2, :], lfm[:, o2, :], RELU,
                             bias=be2c[:, o2:o2 + 1])
    nc.sync.dma_start(out=tout[:, 0:4 * LPC], in_=ostl)

    # level 1: g1 = relu(Wa1^T [leaf_feat; relc] + ba1); max; @Wa2 + ba2
    m1 = []
    for o2 in range(4):
        sl = slice(o2 * 128, (o2 + 1) * 128)
        psA = _agg_psum(f"psA{o2}", o2)[:, 0:LPC]
        for kt in range(4):
            nc.tensor.matmul(psA, wa1[kt][:, sl], ostl[:, kt, :],
                             start=(kt == 0), stop=False)
        nc.tensor.matmul(psA, wa1r[:, sl], relcT, start=False, stop=True)
        g1 = agg.tile([128, LPC], F32, name=f"g1_{o2}", tag=f"g1_{o2}")
        nc.scalar.activation(g1, psA, RELU, bias=ba1c[:, o2:o2 + 1])
        m = agg.tile([128, 1], F32, name=f"m1_{o2}", tag=f"m1_{o2}")
        nc.vector.reduce_max(out=m, in_=g1, axis=mybir.AxisListType.X)
        m1.append(m)

    for o2 in range(4):
        sl = slice(o2 * 128, (o2 + 1) * 128)
        psL = _agg_psum(f"psL{o2}", o2)[:, 0:1]
        for kt in range(4):
            nc.tensor.matmul(psL, wa2[kt][:, sl], m1[kt],
                             start=(kt == 0), stop=(kt == 3))
        nc.scalar.add(osts[:, o2, 0:1], psL, ba2c[:, o2:o2 + 1])

    # root partial: g2 = relu(Wa1^T [lvl1; r2] + ba1) -> host epilogue
    for o2 in range(4):
        sl = slice(o2 * 128, (o2 + 1) * 128)
        psR = _agg_psum(f"psR{o2}", o2)[:, 0:1]
        for kt in range(4):
            nc.tensor.matmul(psR, wa1[kt][:, sl], osts[:, kt, 0:1],
                             start=(kt == 0), stop=False)
        nc.tensor.matmul(psR, wa1r[:, sl], r2T, start=False, stop=True)
        nc.scalar.activation(osts[:, o2, 1:2], psR, RELU,
                             bias=ba1c[:, o2:o2 + 1])
    nc.sync.dma_start(out=tout[:, 4 * LPC:4 * LPC + 8], in_=osts)


_CACHE = {}


def _build(reps=1):
    key = ("nc", reps)
    if key in _CACHE:
        return _CACHE[key]
    nc = bacc.Bacc("TRN2", target_bir_lowering=False, debug=False,
                   num_devices=NCORES)
    shapes = {
        "xT": ((35, PTS), BF16),
        "wsmall": ((128, 158), BF16),
        "wallA": ((128, 512), BF16),
        "wallB": ((128, 1024), BF16),
        "wagg": ((128, 4096), F32),
        "waggr": ((3, 512 + LPC + 1), F32),
    }
    tin = {name: nc.dram_tensor(name, list(shape), dt,
                                kind="ExternalInput").ap()
           for name, (shape, dt) in shapes.items()}
    tout = nc.dram_tensor("out", [128, 4 * OC], F32, kind="ExternalOutput").ap()
    with tile.TileContext(nc) as tc:
        _emit(tc, tin, tout, reps=reps)
    nc.compile()
    _CACHE[key] = nc
    return nc


def _prep_in_maps(inputs):
    f32 = np.float32
    coords = np.asarray(inputs["coords"], f32)
    feats = np.asarray(inputs["feats"], f32)
    leaf_indices = np.asarray(inputs["leaf_indices"])
    leaf_center_idx = np.asarray(inputs["leaf_center_idx"])
    l1_center_idx = np.asarray(inputs["l1_center_idx"])
    root_center_idx = int(np.asarray(inputs["root_center_idx"]))

    pts = coords[leaf_indices]            # (L, K, 3)
    pf = feats[leaf_indices]              # (L, K, C)
    centers = coords[leaf_center_idx]     # (L, 3)
    pp = coords[l1_center_idx]            # (B1, 3)
    rootc = coords[root_center_idx]       # (3,)

    Wp1 = np.asarray(inputs["Wp1"], f32)
    bp1 = np.asarray(inputs["bp1"], f32)
    Wp2 = np.asarray(inputs["Wp2"], f32)
    bp2 = np.asarray(inputs["bp2"], f32)
    We1 = np.asarray(inputs["We1"], f32)
    be1 = np.asarray(inputs["be1"], f32)
    We2 = np.asarray(inputs["We2"], f32)
    be2 = np.asarray(inputs["be2"], f32)
    Wa1 = np.asarray(inputs["Wa1"], f32)
    ba1 = np.asarray(inputs["ba1"], f32)
    Wa2 = np.asarray(inputs["Wa2"], f32)
    ba2 = np.asarray(inputs["ba2"], f32)

    # fold proj's second linear into the encoder first layer (fp64 for safety)
    We1a = np.ascontiguousarray(We1[0:3])                       # (3, 256)
    We1b = We1[3:131]                                           # (128, 256)
    W2e = (Wp2.astype(np.float64) @ We1b.astype(np.float64)).astype(f32)
    be1f = (be1.astype(np.float64)
            + bp2.astype(np.float64) @ We1b.astype(np.float64)).astype(f32)

    biases = np.zeros((128, 15), f32)
    biases[:, 0] = bp1
    biases[:, 1:3] = be1f.reshape(2, 128).T
    biases[:, 3:7] = be2.reshape(4, 128).T
    biases[:, 7:11] = ba1.reshape(4, 128).T
    biases[:, 11:15] = ba2.reshape(4, 128).T

    # small fast-start tensor: f32 biases bitcast into 30 bf16 cols + wp1
    wsmall = np.zeros((128, 158), ml_dtypes.bfloat16)
    wsmall[:, 0:30] = np.ascontiguousarray(biases).view(
        np.uint16).view(ml_dtypes.bfloat16)
    wsmall[0:32, 30:158] = _bf16(Wp1)
    # packed encoder weights: bf16 cols [w2e | we1a] and [we2_0 | we2_1]
    wallA = np.zeros((128, 512), ml_dtypes.bfloat16)
    wallA[:, 0:256] = _bf16(W2e)
    wallA[32:35, 256:512] = _bf16(We1a)
    wallB = np.zeros((128, 1024), ml_dtypes.bfloat16)
    wallB[:, 0:512] = _bf16(We2[0:128])
    wallB[:, 512:1024] = _bf16(We2[128:256])

    wagg = np.zeros((128, 4096), f32)
    for kt in range(4):
        wagg[:, kt * 512:(kt + 1) * 512] = Wa1[kt * 128:(kt + 1) * 128]
        wagg[:, 2048 + kt * 512:2048 + (kt + 1) * 512] = \
            Wa2[kt * 128:(kt + 1) * 128]

    common = {
        "wsmall": np.ascontiguousarray(wsmall),
        "wallA": np.ascontiguousarray(wallA),
        "wallB": np.ascontiguousarray(wallB),
        "wagg": np.ascontiguousarray(wagg),
    }

    in_maps = []
    for m in range(NCORES):
        sl = slice(m * LPC, (m + 1) * LPC)
        im = dict(common)
        xT = np.empty((35, PTS), f32)
        xT[0:32] = pf[sl].reshape(PTS, C).T
        rel = pts[sl] - centers[sl][:, None, :]                 # (LPC, K, 3)
        xT[32:35] = rel.reshape(PTS, 3).T
        im["xT"] = _bf16(xT)
        waggr = np.zeros((3, 512 + LPC + 1), f32)
        waggr[:, 0:512] = Wa1[512:515]
        waggr[:, 512:512 + LPC] = (centers[sl] - pp[m]).T
        waggr[:, 512 + LPC] = pp[m] - rootc
        im["waggr"] = waggr
        in_maps.append(im)
    return in_maps


def _run(inputs, **kwargs):
    nc = _build()
    in_maps = _prep_in_maps(inputs)
    res = run_bass_kernel_spmd(nc, in_maps, core_ids=list(range(NCORES)),
                               **kwargs)
    out = np.empty((1 + NCORES + L, D), np.float32)
    g2 = np.empty((NCORES, D), np.float32)
    for m in range(NCORES):
        o = res.results[m]["out"]                     # (128, 4*LPC + 8)
        leaf = o[:, 0:4 * LPC].reshape(128, 4, LPC)
        out[1 + NCORES + m * LPC:1 + NCORES + (m + 1) * LPC] = \
            leaf.transpose(1, 0, 2).reshape(D, LPC).T
        small = o[:, 4 * LPC:].reshape(128, 4, 2)
        out[1 + m] = small[:, :, 0].T.reshape(D)
        g2[m] = small[:, :, 1].T.reshape(D)
    # root epilogue on host: elementwise max of the per-core g2 partials,
    # then the final 512x512 GEMV.
    Wa2 = np.asarray(inputs["Wa2"], np.float32)
    ba2 = np.asarray(inputs["ba2"], np.float32)
    out[0] = g2.max(axis=0) @ Wa2 + ba2
    return out, res


def kernel(**inputs) -> np.ndarray:
    out, _ = _run(inputs)
    return out


# ---------------------------------------------------------------------------
# dev-only timing helpers (not used by kernel()); safe to keep — they only
# run when called explicitly from test.py.
# ---------------------------------------------------------------------------

def _make_call(nc, in_maps):
    """Build a one-shot timed executor for a compiled bass program."""
    import time

    import jax
    from jax.experimental.shard_map import shard_map
    from jax.sharding import Mesh, NamedSharding, PartitionSpec

    from concourse.bass2jax import (_bass_exec_p, install_neuronx_cc_hook,
                                    partition_id_tensor)

    install_neuronx_cc_hook()
    pname = nc.partition_id_tensor.name if nc.partition_id_tensor else None
    in_names, out_names, out_avals, zero_outs = [], [], [], []
    for alloc in nc.m.functions[0].allocations:
        if not isinstance(alloc, mybir.MemoryLocationSet):
            continue
        name = alloc.memorylocations[0].name
        if alloc.kind == "ExternalInput":
            if name != pname:
                in_names.append(name)
        elif alloc.kind == "ExternalOutput":
            out_names.append(name)
            shape = tuple(alloc.tensor_shape)
            dtype = mybir.dt.np(alloc.dtype)
            out_avals.append(jax.core.ShapedArray(shape, dtype))
            zero_outs.append(np.zeros(shape, dtype))
    n_params = len(in_names)
    all_names = in_names + out_names
    if pname is not None:
        all_names = all_names + [pname]

    def _body(*args):
        operands = list(args)
        if pname is not None:
            operands.append(partition_id_tensor())
        outs = _bass_exec_p.bind(
            *operands, out_avals=tuple(out_avals), in_names=tuple(all_names),
            out_names=tuple(out_names), lowering_input_output_aliases=(),
            sim_require_finite=True, sim_require_nnan=True, nc=nc)
        return tuple(outs)

    ncores = len(in_maps)
    devices = jax.devices()[:ncores]
    mesh = Mesh(np.asarray(devices), ("core",))
    spec = PartitionSpec("core")
    donate = tuple(range(n_params, n_params + len(out_names)))
    fn = jax.jit(
        shard_map(_body, mesh=mesh,
                  in_specs=(spec,) * (n_params + len(out_names)),
                  out_specs=(spec,) * len(out_names), check_rep=False),
        donate_argnums=donate, keep_unused=True)
    sh = NamedSharding(mesh, spec)
    ins = [jax.device_put(
        np.concatenate([np.asarray(m[n]) for m in in_maps], axis=0), sh)
        for n in in_names]
    zs_proto = [np.zeros((ncores * z.shape[0], *z.shape[1:]), z.dtype)
                for z in zero_outs]

    def call():
        zs = [jax.device_put(z, sh) for z in zs_proto]
        jax.block_until_ready(zs)
        t0 = time.perf_counter()
        outs = fn(*ins, *zs)
        # block_until_ready alone does NOT wait for device execution on the
        # axon PJRT stack — a real output fetch does.
        np.asarray(outs[0])
        return time.perf_counter() - t0

    return call


def _pjrt_loop(nc, in_maps, iters):
    call = _make_call(nc, in_maps)
    call()
    return [call() for _ in range(iters)]


def _time_hw(inputs, iters=20, reps=1):
    nc = _build(reps=reps)
    in_maps = _prep_in_maps(inputs)
    return _pjrt_loop(nc, in_maps, iters)


def _time_paired(inputs, reps=10, pairs=100):
    """Per-kernel device time via in-program repetition.

    The e2e call latency here is ~80ms of axon-tunnel overhead with
    multi-ms drift, so a single differential is meaningless.  Instead run
    the 1-rep and `reps`-rep programs interleaved and take the median of
    the per-pair time differences; drift cancels pairwise and the median
    rejects outliers.  Returns (per_rep_ns, pair_diffs_ns).
    """
    import numpy as _np

    in_maps = _prep_in_maps(inputs)
    c1 = _make_call(_build(reps=1), in_maps)
    cR = _make_call(_build(reps=reps), in_maps)
    c1(); cR(); c1(); cR()
    diffs = []
    for i in range(pairs):
        if i % 2 == 0:
            a = c1(); b = cR()
        else:
            b = cR(); a = c1()
        diffs.append((b - a) * 1e9)
    diffs = _np.array(diffs)
    per_rep = float(_np.median(diffs)) / (reps - 1)
    return per_rep, diffs


def _build_baseline():
    if "base" in _CACHE:
        return _CACHE["base"]
    nc = bacc.Bacc("TRN2", target_bir_lowering=False, debug=False,
                   num_devices=NCORES)
    tin = nc.dram_tensor("bx", [128, 4], F32, kind="ExternalInput").ap()
    tout = nc.dram_tensor("bout", [128, 4], F32, kind="ExternalOutput").ap()
    with tile.TileContext(nc) as tc:
        with tc.tile_pool(name="p", bufs=1) as p:
            t = p.tile([128, 4], F32, name="bt", tag="bt")
            nc.sync.dma_start(out=t, in_=tin)
            nc.sync.dma_start(out=tout, in_=t)
    nc.compile()
    _CACHE["base"] = nc
    return nc


def _time_baseline(iters=20):
    nc = _build_baseline()
    in_maps = [{"bx": np.ones((128, 4), np.float32)} for _ in range(NCORES)]
    return _pjrt_loop(nc, in_maps, iters)
